# revision 4
# baseline (speedup 1.0000x reference)
"""Multi-head attention (B=8, N=1024, C=768, 12 heads) on 8 Trainium2 cores.

Strategy: data-parallel over batch — one batch element per NeuronCore, no
collectives. Per core everything stays on-chip:

  1. qkv projection in two orientations:
       - Q^T/K^T tiles [d3, tok]  (lhsT = w_qkv slices, rhs = x^T)
       - V tiles      [tok, d]    (lhsT = x^T slices,  rhs = w_qkv V-columns)
     Only the head-0/1 Q and K chains run up front; the remaining ten chains
     are spread one-per-head through the attention stream so the PE fills its
     exp-wait gaps and the HAM clock-gate stays warm.
  2. attention per head, software-pipelined: per kt-slot the PE-queue order
     is [fillers, PV(kt-2), ST(kt)] so the in-order PE queue never
     head-of-line blocks on ST's PSUM-bank wait (ST(kt) waits for exp(kt-2)
     to drain its bank; fillers and PV are independent of that event).
     Scores S^T[k, q] = K_h Q_h^T on the PE (K=64 contraction streams at
     2 cols/cycle), exp on ScalarE over [128, 1024] tiles (scale=1/8 folded
     in; no max-subtraction — scores are O(5), exp cannot overflow
     fp32/fp16), then PV with a ones-column at position 0 of the V
     stationary so PSUM row 0 is the softmax denominator (rows 1:64 are
     zero padding — PSUM reads must start at partition 0 or 64; rows 64:128
     are the head output). Normalization multiplies rows 64:128 by a
     GpSimd-broadcast reciprocal of row 0.
  3. proj as y^T[c_out, tok] (lhsT = w_proj slices, rhs = attn_out^T tiles,
     per-partition bias); the host transposes back — pure layout.

Inputs are loaded with one batched DMA descriptor per tensor (x^T, w_qkv,
w_proj), issued from three different engine queues so descriptor writes
don't serialize behind each other.

Numerics: all matmul operands fp16 (10-bit mantissa; scores error shrinks by
the 1/8 softmax scale), fp32 PSUM accumulation and fp32 softmax: measured
~7e-4 max rel err vs the fp32 reference. Dummy matmuls ride out the
input-DMA prologue to keep the PE activity monitor at full clock.
"""

import numpy as np

import concourse.bass as bass
import concourse.tile as tile
import concourse.mybir as mybir
from concourse import bacc
from concourse.bass_utils import run_bass_kernel_spmd

dt = mybir.dt
AF = mybir.ActivationFunctionType
ALU = mybir.AluOpType

B = 8
C = 768
N = 1024          # tokens per batch element (32*32)
NH = 12           # heads
HD = 64           # head dim
C3 = 3 * C        # 2304
CT = C // 128     # 6 contraction tiles
TT = N // 128     # 8 token tiles
NQH = 2           # q processed in halves of 512 where PSUM-bank-bound
QHW = N // NQH    # 512
SCALE = HD ** -0.5
N_WARMUP_MM = 55  # dummy matmuls riding out the input-DMA prologue


def _build_nc():
    nc = bacc.Bacc(None, target_bir_lowering=False)

    xt_ext = nc.dram_tensor("xt", [C, N], dt.float16, kind="ExternalInput")
    wq_ext = nc.dram_tensor("w_qkv", [C, C3], dt.float16, kind="ExternalInput")
    bqk_ext = nc.dram_tensor("b_qkt", [128, 2 * C // 128], dt.float32, kind="ExternalInput")
    bv_ext = nc.dram_tensor("b_v", [1, C], dt.float32, kind="ExternalInput")
    wp_ext = nc.dram_tensor("w_proj", [C, C], dt.float16, kind="ExternalInput")
    bp_ext = nc.dram_tensor("b_proj", [128, C // 128], dt.float32, kind="ExternalInput")
    y_ext = nc.dram_tensor("y", [C, N], dt.float32, kind="ExternalOutput")

    with (
        tile.TileContext(nc) as tc,
        tc.tile_pool(name="persist", bufs=1) as pp,
        tc.tile_pool(name="qkrot", bufs=3) as qkrot,
        tc.tile_pool(name="ps_big", bufs=2, space="PSUM") as ps_big,
        tc.tile_pool(name="ps_o", bufs=4, space="PSUM") as ps_o,
        tc.tile_pool(name="att_sb", bufs=12) as att_sb,
        tc.tile_pool(name="att_small", bufs=3) as att_small,
        tc.tile_pool(name="y_sb", bufs=3) as y_pool,
    ):
        # ---- constants / biases ----
        ones_f32 = pp.tile([128, NH, 1], dt.float32, tag="ones_f32")
        nc.vector.memset(ones_f32[:], 1.0)
        bqk_sb = pp.tile([128, 2 * C // 128], dt.float32, tag="bqk")
        nc.gpsimd.dma_start(out=bqk_sb[:], in_=bqk_ext[:, :])
        bv_sb = pp.tile([128, C], dt.float32, tag="bv")
        nc.gpsimd.dma_start(out=bv_sb[:], in_=bv_ext[0:1, :].to_broadcast((128, C)))
        bp_sb = pp.tile([128, C // 128], dt.float32, tag="bp")
        nc.gpsimd.dma_start(out=bp_sb[:], in_=bp_ext[:, :])

        dummy_sb = pp.tile([128, QHW], dt.float16, tag="dummy_sb")
        nc.vector.memset(dummy_sb[:].bitcast(dt.float32), 0.0)

        # ---- batched input loads: one descriptor per tensor, three queues ----
        xt_all = pp.tile([128, CT, N], dt.float16, tag="xt_all")
        nc.gpsimd.dma_start(
            out=xt_all[:], in_=xt_ext[:, :].rearrange("(i p) n -> p i n", p=128)
        )
        wq_all = pp.tile([128, CT, C3], dt.float16, tag="wq_all")
        nc.sync.dma_start(
            out=wq_all[:], in_=wq_ext[:, :].rearrange("(i p) n -> p i n", p=128)
        )
        wp_all = pp.tile([128, CT, C], dt.float16, tag="wp_all")
        nc.scalar.dma_start(
            out=wp_all[:], in_=wp_ext[:, :].rearrange("(i p) n -> p i n", p=128)
        )
        xt_sb = [xt_all[:, i, :] for i in range(CT)]
        wq_sb = [wq_all[:, i, :] for i in range(CT)]
        wp_sb = [wp_all[:, i, :] for i in range(CT)]

        # attn_out^T: 6 persistent tiles of [128, N]
        aT = [pp.tile([128, N], dt.float16, name=f"aT{i}", tag=f"aT{i}") for i in range(CT)]
        # V per token-tile [128, NH, 128] fp16: col 0 = ones (softmax
        # denominator row), cols 1:64 zero, cols 64:128 = V for that head.
        v_sb = [pp.tile([128, NH, 128], dt.float16, name=f"v{i}", tag=f"v{i}") for i in range(TT)]

        # rotating Q^T/K^T tiles, keyed by d3-tile index
        qkT = {}

        def qk_chain_thunks(d3):
            """Build one Q^T/K^T projection chain as a list of thunks (12
            matmuls into two half-bank psum chains, then bias -> qkT[d3]) so
            the matmuls can be sprinkled into the attention PE stream."""
            t = qkrot.tile([128, N], dt.float16, tag="qv" if d3 < 6 else "kv",
                           name=f"qkT{d3}")
            qkT[d3] = t
            pss = [ps_o.tile([128, QHW], dt.float32, tag="pov", name=f"ps_qk{d3}_{qh}")
                   for qh in range(NQH)]
            thunks = []

            def mk_mm(qh, ct):
                def run():
                    nc.tensor.matmul(
                        out=pss[qh][:],
                        lhsT=wq_sb[ct][:, 128 * d3:128 * (d3 + 1)],
                        rhs=xt_sb[ct][:, QHW * qh:QHW * (qh + 1)],
                        start=(ct == 0), stop=(ct == CT - 1),
                    )
                return run

            for qh in range(NQH):
                for ct in range(CT):
                    thunks.append(mk_mm(qh, ct))

            def bias():
                for qh in range(NQH):
                    nc.vector.tensor_scalar(
                        out=t[:, QHW * qh:QHW * (qh + 1)], in0=pss[qh][:],
                        scalar1=bqk_sb[:, d3:d3 + 1], scalar2=None, op0=ALU.add,
                    )
            thunks.append(bias)
            return thunks

        def qk_chain(d3):
            for th in qk_chain_thunks(d3):
                th()

        def att_head(h, fillers=()):
            q_tile = qkT[h // 2]
            k_tile = qkT[6 + h // 2]
            po = 64 * (h % 2)
            fillers = list(fillers)
            # Software-pipelined head: per kt-slot emit fillers, then
            # PV(kt-2), then ST(kt)+exp(kt). ST waits on exp(kt-2)'s PSUM
            # bank; emitting the independent work first keeps the in-order
            # PE queue busy through that wait.
            PIPE = 2
            ess = []
            povs = []
            for qh in range(NQH):
                pov = ps_o.tile([128, QHW], dt.float32, tag="pov", name=f"pov{h}_{qh}")
                povs.append(pov)
            fi = 0
            NSLOT = TT + PIPE

            def pv_pair(kt):
                for qh in range(NQH):
                    nc.tensor.matmul(
                        out=povs[qh][:],
                        lhsT=v_sb[kt][:, h, :],
                        rhs=ess[kt][:, QHW * qh:QHW * (qh + 1)],
                        start=(kt == 0), stop=(kt == TT - 1),
                    )

            for kt in range(NSLOT):
                # fillers first: anything a PV may consume (e.g. att0's
                # V tiles) must be emitted before the PV that reads it
                want = min(len(fillers), -(-((kt + 1) * len(fillers)) // NSLOT))
                while fi < want:
                    fillers[fi]()
                    fi += 1
                if kt >= PIPE:
                    pv_pair(kt - PIPE)
                if kt < TT:
                    pss = ps_big.tile([128, N], dt.float32, tag="big", name=f"pss{h}_{kt}")
                    for qh in range(NQH):
                        nc.tensor.matmul(
                            out=pss[:, QHW * qh:QHW * (qh + 1)],
                            lhsT=k_tile[po:po + HD, 128 * kt:128 * (kt + 1)],
                            rhs=q_tile[po:po + HD, QHW * qh:QHW * (qh + 1)],
                            start=True, stop=True,
                        )
                    es = att_sb.tile([128, N], dt.float16, tag="es", name=f"es{h}_{kt}")
                    nc.scalar.activation(out=es[:], in_=pss[:], func=AF.Exp, scale=SCALE)
                    ess.append(es)
            while fi < len(fillers):
                fillers[fi]()
                fi += 1
            # normalize rows 64:128 by reciprocal of denominator row 0
            for qh in range(NQH):
                r_sb = att_small.tile([1, QHW], dt.float32, tag="r")
                nc.vector.reciprocal_approx_fast(out=r_sb[:], in_=povs[qh][0:1, :])
                rb_sb = att_small.tile([HD, QHW], dt.float32, tag="rb")
                nc.gpsimd.partition_broadcast(rb_sb[:], r_sb[:])
                nc.vector.tensor_tensor(
                    out=aT[h // 2][po:po + HD, QHW * qh:QHW * (qh + 1)],
                    in0=povs[qh][64:128, :],
                    in1=rb_sb[:],
                    op=ALU.mult,
                )

        # HAM warm-up: dummy matmuls with no input dependencies
        pwarm = ps_big.tile([128, QHW], dt.float32, tag="big", name="pwarm")
        for _ in range(N_WARMUP_MM):
            nc.tensor.matmul(
                out=pwarm[:], lhsT=dummy_sb[:, 0:128], rhs=dummy_sb[:],
                start=True, stop=True,
            )

        qk_chain(0)   # Q heads 0/1
        qk_chain(6)   # K heads 0/1

        # V part of the qkv projection, as thunk lists
        def v_chain_thunks(tt):
            ps = ps_big.tile([128, N], dt.float32, tag="big", name=f"ps_v{tt}")
            thunks = []

            def mk_mm(c0, c1, ct):
                def run():
                    nc.tensor.matmul(
                        out=ps[:, c0:c1],
                        lhsT=xt_sb[ct][:, 128 * tt:128 * (tt + 1)],
                        rhs=wq_sb[ct][:, 2 * C + c0:2 * C + c1],
                        start=(ct == 0), stop=(ct == CT - 1),
                    )
                return run

            for c0, c1 in ((0, 512), (512, C)):
                for ct in range(CT):
                    thunks.append(mk_mm(c0, c1, ct))

            def finish():
                nc.vector.memset(v_sb[tt][:].bitcast(dt.float32), 0.0)
                nc.vector.tensor_tensor(
                    out=v_sb[tt][:, :, 64:128],
                    in0=ps[:, 0:C].rearrange("p (h d) -> p h d", h=NH),
                    in1=bv_sb[:].rearrange("p (h d) -> p h d", h=NH),
                    op=ALU.add,
                )
                nc.vector.tensor_copy(out=v_sb[tt][:, :, 0:1], in_=ones_f32[:])
            thunks.append(finish)
            return thunks

        # V tiles 0..1 up front; V 2..7 ride att0's filler slots
        for tt in range(2):
            for th in v_chain_thunks(tt):
                th()

        # attention heads 0..11 with remaining qkv work sprinkled into
        # each head's PE stream: att0 carries V tiles 2..7; att(h) for
        # h=1..9 carries one Q/K chain (pair j delivered during heads
        # 2j-1 and 2j, consumed from head 2j+2 on)
        filler_plan = {
            0: lambda: [t for tt in range(2, TT) for t in v_chain_thunks(tt)],
            1: lambda: qk_chain_thunks(1) + qk_chain_thunks(7),
            2: lambda: qk_chain_thunks(2),
            3: lambda: qk_chain_thunks(8),
            4: lambda: qk_chain_thunks(3),
            5: lambda: qk_chain_thunks(9),
            6: lambda: qk_chain_thunks(4),
            7: lambda: qk_chain_thunks(10),
            8: lambda: qk_chain_thunks(5),
            9: lambda: qk_chain_thunks(11),
        }
        for h in range(NH):
            att_head(h, filler_plan[h]() if h in filler_plan else ())

        # ---- output projection (y^T: [c_out, tok]; host untransposes) ----
        for co in range(CT):
            ps = ps_big.tile([128, N], dt.float32, tag="big", name=f"ps_y{co}")
            for qh in range(NQH):
                for ct in range(CT):
                    nc.tensor.matmul(
                        out=ps[:, QHW * qh:QHW * (qh + 1)],
                        lhsT=wp_sb[ct][:, 128 * co:128 * (co + 1)],
                        rhs=aT[ct][:, QHW * qh:QHW * (qh + 1)],
                        start=(ct == 0), stop=(ct == CT - 1),
                    )
            y_sb = y_pool.tile([128, N], dt.float32, tag="y")
            nc.vector.tensor_scalar(
                out=y_sb[:], in0=ps[:],
                scalar1=bp_sb[:, co:co + 1], scalar2=None, op0=ALU.add,
            )
            nc.sync.dma_start(out=y_ext[128 * co:128 * (co + 1), :], in_=y_sb[:])

    nc.compile()
    return nc


_NC_CACHE = {}


def kernel(x, w_qkv, b_qkv, w_proj, b_proj, _trace=False):
    x = np.asarray(x, dtype=np.float32)
    w_qkv = np.asarray(w_qkv, dtype=np.float32)
    b_qkv = np.asarray(b_qkv, dtype=np.float32)
    w_proj = np.asarray(w_proj, dtype=np.float32)
    b_proj = np.asarray(b_proj, dtype=np.float32)

    if "nc" not in _NC_CACHE:
        _NC_CACHE["nc"] = _build_nc()
    nc = _NC_CACHE["nc"]

    # host-side prep (pure layout, no arithmetic)
    # b_qkt: Q/K bias columns laid out per d3-tile: [128, 12]
    b_qkt = np.ascontiguousarray(b_qkv[:2 * C].reshape(2 * C // 128, 128).T)
    w_qkv_h = w_qkv.astype(np.float16)
    w_proj_h = w_proj.astype(np.float16)
    b_v = np.ascontiguousarray(b_qkv[2 * C:].reshape(1, C))
    b_p = np.ascontiguousarray(b_proj.reshape(C // 128, 128).T)

    core_ids = list(range(B))
    in_maps = []
    for b in range(B):
        xt = np.ascontiguousarray(x[b].reshape(N, C).T.astype(np.float16))
        in_maps.append({
            "xt": xt,
            "w_qkv": w_qkv_h,
            "b_qkt": b_qkt,
            "b_v": b_v,
            "w_proj": w_proj_h,
            "b_proj": b_p,
        })

    res = run_bass_kernel_spmd(nc, in_maps, core_ids, trace=_trace)
    if _trace:
        _NC_CACHE["last_result"] = res

    out = np.empty((B, 32, 32, C), dtype=np.float32)
    for b in range(B):
        out[b] = res.results[b]["y"].T.reshape(32, 32, C)
    return out


# revision 7
# speedup vs baseline: 1.2072x; 1.2072x over previous
"""Multi-head attention (B=8, N=1024, C=768, 12 heads) on 8 Trainium2 cores.

Strategy: data-parallel over batch — one batch element per NeuronCore, no
collectives. Per core everything stays on-chip:

  1. qkv projection in two orientations:
       - Q^T/K^T tiles [d3, tok]  (lhsT = w_qkv slices, rhs = x^T)
       - V tiles      [tok, d]    (lhsT = x^T slices,  rhs = w_qkv V-columns)
     Only the head-0/1 Q and K chains run up front; the remaining ten chains
     are spread one-per-head through the attention stream so the PE fills its
     exp-wait gaps and the HAM clock-gate stays warm.
  2. attention per head, software-pipelined: per kt-slot the PE-queue order
     is [fillers, PV(kt-2), ST(kt)] so the in-order PE queue never
     head-of-line blocks on ST's PSUM-bank wait (ST(kt) waits for exp(kt-2)
     to drain its bank; fillers and PV are independent of that event).
     Scores S^T[k, q] = K_h Q_h^T on the PE (K=64 contraction streams at
     2 cols/cycle), exp on ScalarE over [128, 1024] tiles (scale=1/8 folded
     in; no max-subtraction — scores are O(5), exp cannot overflow
     fp32/fp16), then PV with a ones-column at position 0 of the V
     stationary so PSUM row 0 is the softmax denominator (rows 1:64 are
     zero padding — PSUM reads must start at partition 0 or 64; rows 64:128
     are the head output). Normalization multiplies rows 64:128 by a
     GpSimd-broadcast reciprocal of row 0.
  3. proj as y^T[c_out, tok] (lhsT = w_proj slices, rhs = attn_out^T tiles,
     per-partition bias); the host transposes back — pure layout.

Inputs are loaded with one batched DMA descriptor per tensor (x^T, w_qkv,
w_proj), issued from three different engine queues so descriptor writes
don't serialize behind each other.

Numerics: all matmul operands fp16 (10-bit mantissa; scores error shrinks by
the 1/8 softmax scale), fp32 PSUM accumulation and fp32 softmax: measured
~7e-4 max rel err vs the fp32 reference. Dummy matmuls ride out the
input-DMA prologue to keep the PE activity monitor at full clock.
"""

import numpy as np

import concourse.bass as bass
import concourse.tile as tile
import concourse.mybir as mybir
from concourse import bacc
from concourse.bass_utils import run_bass_kernel_spmd

dt = mybir.dt
AF = mybir.ActivationFunctionType
ALU = mybir.AluOpType

B = 8
C = 768
N = 1024          # tokens per batch element (32*32)
NH = 12           # heads
HD = 64           # head dim
C3 = 3 * C        # 2304
CT = C // 128     # 6 contraction tiles
TT = N // 128     # 8 token tiles
NQH = 2           # q processed in halves of 512 where PSUM-bank-bound
QHW = N // NQH    # 512
SCALE = HD ** -0.5
N_WARMUP_MM = 30  # dummy matmuls riding out the input-DMA prologue


def _build_nc():
    nc = bacc.Bacc(None, target_bir_lowering=False)

    xt_ext = nc.dram_tensor("xt", [C, N], dt.float16, kind="ExternalInput")
    wq_ext = nc.dram_tensor("w_qkv", [C, C3], dt.float16, kind="ExternalInput")
    bqk_ext = nc.dram_tensor("b_qkt", [128, 2 * C // 128], dt.float32, kind="ExternalInput")
    bv_ext = nc.dram_tensor("b_v", [1, C], dt.float32, kind="ExternalInput")
    wp_ext = nc.dram_tensor("w_proj", [C, C], dt.float16, kind="ExternalInput")
    bp_ext = nc.dram_tensor("b_proj", [128, C // 128], dt.float32, kind="ExternalInput")
    y_ext = nc.dram_tensor("y", [C, N], dt.float32, kind="ExternalOutput")

    with (
        tile.TileContext(nc) as tc,
        tc.tile_pool(name="persist", bufs=1) as pp,
        tc.tile_pool(name="qkrot", bufs=3) as qkrot,
        tc.tile_pool(name="ps_big", bufs=2, space="PSUM") as ps_big,
        tc.tile_pool(name="ps_o", bufs=4, space="PSUM") as ps_o,
        tc.tile_pool(name="att_sb", bufs=12) as att_sb,
        tc.tile_pool(name="att_small", bufs=3) as att_small,
        tc.tile_pool(name="y_sb", bufs=3) as y_pool,
    ):
        # ---- constants / biases ----
        ones_f32 = pp.tile([128, NH, 1], dt.float32, tag="ones_f32")
        nc.vector.memset(ones_f32[:], 1.0)
        bqk_sb = pp.tile([128, 2 * C // 128], dt.float32, tag="bqk")
        nc.gpsimd.dma_start(out=bqk_sb[:], in_=bqk_ext[:, :])
        bv_sb = pp.tile([128, C], dt.float32, tag="bv")
        nc.gpsimd.dma_start(out=bv_sb[:], in_=bv_ext[0:1, :].to_broadcast((128, C)))
        bp_sb = pp.tile([128, C // 128], dt.float32, tag="bp")
        nc.gpsimd.dma_start(out=bp_sb[:], in_=bp_ext[:, :])

        dummy_sb = pp.tile([128, QHW], dt.float16, tag="dummy_sb")
        nc.vector.memset(dummy_sb[:].bitcast(dt.float32), 0.0)

        # ---- batched input loads: one descriptor per tensor, three queues ----
        xt_all = pp.tile([128, CT, N], dt.float16, tag="xt_all")
        nc.gpsimd.dma_start(
            out=xt_all[:], in_=xt_ext[:, :].rearrange("(i p) n -> p i n", p=128)
        )
        # wq per-ct on the sync queue: the first qk chain only needs tile 0
        # to start, so one completion event per tile beats one big event.
        wq_all = pp.tile([128, CT, C3], dt.float16, tag="wq_all")
        for i in range(CT):
            nc.sync.dma_start(
                out=wq_all[:, i, :], in_=wq_ext[128 * i:128 * (i + 1), :]
            )
        wp_all = pp.tile([128, CT, C], dt.float16, tag="wp_all")
        nc.scalar.dma_start(
            out=wp_all[:], in_=wp_ext[:, :].rearrange("(i p) n -> p i n", p=128)
        )
        xt_sb = [xt_all[:, i, :] for i in range(CT)]
        wq_sb = [wq_all[:, i, :] for i in range(CT)]
        wp_sb = [wp_all[:, i, :] for i in range(CT)]

        # attn_out^T: 6 persistent tiles of [128, N]
        aT = [pp.tile([128, N], dt.float16, name=f"aT{i}", tag=f"aT{i}") for i in range(CT)]
        # V per token-tile [128, NH, 128] fp16: col 0 = ones (softmax
        # denominator row), cols 1:64 zero, cols 64:128 = V for that head.
        v_sb = [pp.tile([128, NH, 128], dt.float16, name=f"v{i}", tag=f"v{i}") for i in range(TT)]

        # rotating Q^T/K^T tiles, keyed by d3-tile index
        qkT = {}

        def qk_chain_thunks(d3):
            """Build one Q^T/K^T projection chain as a list of thunks (12
            matmuls into two half-bank psum chains, then bias -> qkT[d3]) so
            the matmuls can be sprinkled into the attention PE stream."""
            t = qkrot.tile([128, N], dt.float16, tag="qv" if d3 < 6 else "kv",
                           name=f"qkT{d3}")
            qkT[d3] = t
            pss = [ps_o.tile([128, QHW], dt.float32, tag="pov", name=f"ps_qk{d3}_{qh}")
                   for qh in range(NQH)]
            thunks = []

            def mk_mm(qh, ct):
                def run():
                    nc.tensor.matmul(
                        out=pss[qh][:],
                        lhsT=wq_sb[ct][:, 128 * d3:128 * (d3 + 1)],
                        rhs=xt_sb[ct][:, QHW * qh:QHW * (qh + 1)],
                        start=(ct == 0), stop=(ct == CT - 1),
                    )
                return run

            for qh in range(NQH):
                for ct in range(CT):
                    thunks.append(mk_mm(qh, ct))

            def bias():
                for qh in range(NQH):
                    nc.vector.tensor_scalar(
                        out=t[:, QHW * qh:QHW * (qh + 1)], in0=pss[qh][:],
                        scalar1=bqk_sb[:, d3:d3 + 1], scalar2=None, op0=ALU.add,
                    )
            thunks.append(bias)
            return thunks

        def qk_chain(d3):
            for th in qk_chain_thunks(d3):
                th()

        def att_head(h, fillers=()):
            q_tile = qkT[h // 2]
            k_tile = qkT[6 + h // 2]
            po = 64 * (h % 2)
            fillers = list(fillers)
            # Software-pipelined head: per kt-slot emit fillers, then
            # PV(kt-2), then ST(kt)+exp(kt). ST waits on exp(kt-2)'s PSUM
            # bank; emitting the independent work first keeps the in-order
            # PE queue busy through that wait.
            PIPE = 2
            ess = []
            povs = []
            for qh in range(NQH):
                pov = ps_o.tile([128, QHW], dt.float32, tag="pov", name=f"pov{h}_{qh}")
                povs.append(pov)
            fi = 0
            # 2 kt per slot: the K=64 score matmuls run in the PE's 64x128
            # tiling mode while PV/fillers run 128x128 — batching two kt of
            # STs (and two kt of PVs) per slot halves the mode-switch drains.
            NSLOT = TT // 2 + PIPE

            def pv_pair(kt):
                for qh in range(NQH):
                    nc.tensor.matmul(
                        out=povs[qh][:],
                        lhsT=v_sb[kt][:, h, :],
                        rhs=ess[kt][:, QHW * qh:QHW * (qh + 1)],
                        start=(kt == 0), stop=(kt == TT - 1),
                    )

            for sl in range(NSLOT):
                # fillers first: anything a PV may consume (e.g. att0's
                # V tiles) must be emitted before the PV that reads it
                want = min(len(fillers), -(-((sl + 1) * len(fillers)) // NSLOT))
                while fi < want:
                    fillers[fi]()
                    fi += 1
                if sl >= PIPE:
                    pv_pair(2 * (sl - PIPE))
                    pv_pair(2 * (sl - PIPE) + 1)
                if sl < TT // 2:
                    pair = []
                    for kt in (2 * sl, 2 * sl + 1):
                        pss = ps_big.tile([128, N], dt.float32, tag="big", name=f"pss{h}_{kt}")
                        for qh in range(NQH):
                            nc.tensor.matmul(
                                out=pss[:, QHW * qh:QHW * (qh + 1)],
                                lhsT=k_tile[po:po + HD, 128 * kt:128 * (kt + 1)],
                                rhs=q_tile[po:po + HD, QHW * qh:QHW * (qh + 1)],
                                start=True, stop=True,
                            )
                        pair.append(pss)
                    for kt in (2 * sl, 2 * sl + 1):
                        es = att_sb.tile([128, N], dt.float16, tag="es", name=f"es{h}_{kt}")
                        nc.scalar.activation(
                            out=es[:], in_=pair[kt - 2 * sl][:], func=AF.Exp, scale=SCALE
                        )
                        ess.append(es)
            while fi < len(fillers):
                fillers[fi]()
                fi += 1
            # normalize rows 64:128 by reciprocal of denominator row 0
            for qh in range(NQH):
                r_sb = att_small.tile([1, QHW], dt.float32, tag="r")
                nc.vector.reciprocal_approx_fast(out=r_sb[:], in_=povs[qh][0:1, :])
                rb_sb = att_small.tile([HD, QHW], dt.float32, tag="rb")
                nc.gpsimd.partition_broadcast(rb_sb[:], r_sb[:])
                nc.vector.tensor_tensor(
                    out=aT[h // 2][po:po + HD, QHW * qh:QHW * (qh + 1)],
                    in0=povs[qh][64:128, :],
                    in1=rb_sb[:],
                    op=ALU.mult,
                )

        # HAM warm-up: dummy matmuls with no input dependencies
        pwarm = ps_big.tile([128, QHW], dt.float32, tag="big", name="pwarm")
        for _ in range(N_WARMUP_MM):
            nc.tensor.matmul(
                out=pwarm[:], lhsT=dummy_sb[:, 0:128], rhs=dummy_sb[:],
                start=True, stop=True,
            )

        qk_chain(0)   # Q heads 0/1
        qk_chain(6)   # K heads 0/1

        # V part of the qkv projection, as thunk lists
        def v_chain_thunks(tt):
            ps = ps_big.tile([128, N], dt.float32, tag="big", name=f"ps_v{tt}")
            thunks = []

            def mk_mm(c0, c1, ct):
                def run():
                    nc.tensor.matmul(
                        out=ps[:, c0:c1],
                        lhsT=xt_sb[ct][:, 128 * tt:128 * (tt + 1)],
                        rhs=wq_sb[ct][:, 2 * C + c0:2 * C + c1],
                        start=(ct == 0), stop=(ct == CT - 1),
                    )
                return run

            for c0, c1 in ((0, 512), (512, C)):
                for ct in range(CT):
                    thunks.append(mk_mm(c0, c1, ct))

            def finish():
                nc.vector.memset(v_sb[tt][:].bitcast(dt.float32), 0.0)
                nc.vector.tensor_tensor(
                    out=v_sb[tt][:, :, 64:128],
                    in0=ps[:, 0:C].rearrange("p (h d) -> p h d", h=NH),
                    in1=bv_sb[:].rearrange("p (h d) -> p h d", h=NH),
                    op=ALU.add,
                )
                nc.vector.tensor_copy(out=v_sb[tt][:, :, 0:1], in_=ones_f32[:])
            thunks.append(finish)
            return thunks

        # V tiles 0..1 up front; V 2..7 ride att0's filler slots
        for tt in range(2):
            for th in v_chain_thunks(tt):
                th()

        # attention heads 0..11 with remaining qkv work sprinkled into
        # each head's PE stream: att0 carries V tiles 2..7; att(h) for
        # h=1..9 carries one Q/K chain (pair j delivered during heads
        # 2j-1 and 2j, consumed from head 2j+2 on)
        filler_plan = {
            0: lambda: [t for tt in range(2, TT) for t in v_chain_thunks(tt)],
            1: lambda: qk_chain_thunks(1) + qk_chain_thunks(7),
            2: lambda: qk_chain_thunks(2),
            3: lambda: qk_chain_thunks(8),
            4: lambda: qk_chain_thunks(3),
            5: lambda: qk_chain_thunks(9),
            6: lambda: qk_chain_thunks(4),
            7: lambda: qk_chain_thunks(10),
            8: lambda: qk_chain_thunks(5),
            9: lambda: qk_chain_thunks(11),
        }
        for h in range(NH):
            att_head(h, filler_plan[h]() if h in filler_plan else ())

        # ---- output projection (y^T: [c_out, tok]; host untransposes) ----
        for co in range(CT):
            ps = ps_big.tile([128, N], dt.float32, tag="big", name=f"ps_y{co}")
            for qh in range(NQH):
                for ct in range(CT):
                    nc.tensor.matmul(
                        out=ps[:, QHW * qh:QHW * (qh + 1)],
                        lhsT=wp_sb[ct][:, 128 * co:128 * (co + 1)],
                        rhs=aT[ct][:, QHW * qh:QHW * (qh + 1)],
                        start=(ct == 0), stop=(ct == CT - 1),
                    )
            y_sb = y_pool.tile([128, N], dt.float32, tag="y")
            nc.vector.tensor_scalar(
                out=y_sb[:], in0=ps[:],
                scalar1=bp_sb[:, co:co + 1], scalar2=None, op0=ALU.add,
            )
            nc.sync.dma_start(out=y_ext[128 * co:128 * (co + 1), :], in_=y_sb[:])

    nc.compile()
    return nc


_NC_CACHE = {}


def kernel(x, w_qkv, b_qkv, w_proj, b_proj, _trace=False):
    x = np.asarray(x, dtype=np.float32)
    w_qkv = np.asarray(w_qkv, dtype=np.float32)
    b_qkv = np.asarray(b_qkv, dtype=np.float32)
    w_proj = np.asarray(w_proj, dtype=np.float32)
    b_proj = np.asarray(b_proj, dtype=np.float32)

    if "nc" not in _NC_CACHE:
        _NC_CACHE["nc"] = _build_nc()
    nc = _NC_CACHE["nc"]

    # host-side prep (pure layout, no arithmetic)
    # b_qkt: Q/K bias columns laid out per d3-tile: [128, 12]
    b_qkt = np.ascontiguousarray(b_qkv[:2 * C].reshape(2 * C // 128, 128).T)
    w_qkv_h = w_qkv.astype(np.float16)
    w_proj_h = w_proj.astype(np.float16)
    b_v = np.ascontiguousarray(b_qkv[2 * C:].reshape(1, C))
    b_p = np.ascontiguousarray(b_proj.reshape(C // 128, 128).T)

    core_ids = list(range(B))
    in_maps = []
    for b in range(B):
        xt = np.ascontiguousarray(x[b].reshape(N, C).T.astype(np.float16))
        in_maps.append({
            "xt": xt,
            "w_qkv": w_qkv_h,
            "b_qkt": b_qkt,
            "b_v": b_v,
            "w_proj": w_proj_h,
            "b_proj": b_p,
        })

    res = run_bass_kernel_spmd(nc, in_maps, core_ids, trace=_trace)
    if _trace:
        _NC_CACHE["last_result"] = res

    out = np.empty((B, 32, 32, C), dtype=np.float32)
    for b in range(B):
        out[b] = res.results[b]["y"].T.reshape(32, 32, C)
    return out


# revision 9
# speedup vs baseline: 1.2670x; 1.0496x over previous
"""Multi-head attention (B=8, N=1024, C=768, 12 heads) on 8 Trainium2 cores.

Strategy: data-parallel over batch — one batch element per NeuronCore, no
collectives. Per core everything stays on-chip:

  1. qkv projection in two orientations:
       - Q^T/K^T tiles [d3, tok]  (lhsT = w_qkv slices, rhs = x^T)
       - V tiles      [tok, d]    (lhsT = x^T slices,  rhs = w_qkv V-columns)
     Only the head-0/1 Q and K chains run up front; the remaining ten chains
     are spread one-per-head through the attention stream so the PE fills its
     exp-wait gaps and the HAM clock-gate stays warm.
  2. attention per head, software-pipelined: per kt-slot the PE-queue order
     is [fillers, PV(kt-2), ST(kt)] so the in-order PE queue never
     head-of-line blocks on ST's PSUM-bank wait (ST(kt) waits for exp(kt-2)
     to drain its bank; fillers and PV are independent of that event).
     Scores S^T[k, q] = K_h Q_h^T on the PE (K=64 contraction streams at
     2 cols/cycle), exp on ScalarE over [128, 1024] tiles (scale=1/8 folded
     in; no max-subtraction — scores are O(5), exp cannot overflow
     fp32/fp16), then PV with a ones-column at position 0 of the V
     stationary so PSUM row 0 is the softmax denominator (rows 1:64 are
     zero padding — PSUM reads must start at partition 0 or 64; rows 64:128
     are the head output). Normalization multiplies rows 64:128 by a
     GpSimd-broadcast reciprocal of row 0.
  3. proj as y^T[c_out, tok] (lhsT = w_proj slices, rhs = attn_out^T tiles,
     per-partition bias); the host transposes back — pure layout.

Inputs are loaded with one batched DMA descriptor per tensor (x^T, w_qkv,
w_proj), issued from three different engine queues so descriptor writes
don't serialize behind each other.

Numerics: all matmul operands fp16 (10-bit mantissa; scores error shrinks by
the 1/8 softmax scale), fp32 PSUM accumulation and fp32 softmax: measured
~7e-4 max rel err vs the fp32 reference. Dummy matmuls ride out the
input-DMA prologue to keep the PE activity monitor at full clock.
"""

import numpy as np

import concourse.bass as bass
import concourse.tile as tile
import concourse.mybir as mybir
from concourse import bacc
from concourse.bass_utils import run_bass_kernel_spmd

dt = mybir.dt
AF = mybir.ActivationFunctionType
ALU = mybir.AluOpType

B = 8
C = 768
N = 1024          # tokens per batch element (32*32)
NH = 12           # heads
HD = 64           # head dim
C3 = 3 * C        # 2304
CT = C // 128     # 6 contraction tiles
TT = N // 128     # 8 token tiles
NQH = 2           # q processed in halves of 512 where PSUM-bank-bound
QHW = N // NQH    # 512
SCALE = HD ** -0.5
N_WARMUP_MM = 40  # dummy matmuls riding out the input-DMA prologue


def _build_nc():
    nc = bacc.Bacc(None, target_bir_lowering=False)

    xt_ext = nc.dram_tensor("xt", [C, N], dt.float16, kind="ExternalInput")
    wq_ext = nc.dram_tensor("w_qkv", [C, C3], dt.float16, kind="ExternalInput")
    bqk_ext = nc.dram_tensor("b_qkt", [128, 2 * C // 128], dt.float32, kind="ExternalInput")
    bv_ext = nc.dram_tensor("b_v", [1, C], dt.float32, kind="ExternalInput")
    wp_ext = nc.dram_tensor("w_proj", [C, C], dt.float16, kind="ExternalInput")
    bp_ext = nc.dram_tensor("b_proj", [128, C // 128], dt.float32, kind="ExternalInput")
    y_ext = nc.dram_tensor("y", [C, N], dt.float16, kind="ExternalOutput")

    with (
        tile.TileContext(nc) as tc,
        tc.tile_pool(name="persist", bufs=1) as pp,
        tc.tile_pool(name="qkrot", bufs=3) as qkrot,
        tc.tile_pool(name="ps_big", bufs=2, space="PSUM") as ps_big,
        tc.tile_pool(name="ps_o", bufs=4, space="PSUM") as ps_o,
        tc.tile_pool(name="att_sb", bufs=12) as att_sb,
        tc.tile_pool(name="att_small", bufs=3) as att_small,
        tc.tile_pool(name="y_sb", bufs=3) as y_pool,
    ):
        # ---- constants / biases ----
        ones_f32 = pp.tile([128, NH, 1], dt.float32, tag="ones_f32")
        nc.vector.memset(ones_f32[:], 1.0)
        bqk_sb = pp.tile([128, 2 * C // 128], dt.float32, tag="bqk")
        nc.gpsimd.dma_start(out=bqk_sb[:], in_=bqk_ext[:, :])
        bv_sb = pp.tile([128, C], dt.float32, tag="bv")
        nc.gpsimd.dma_start(out=bv_sb[:], in_=bv_ext[0:1, :].to_broadcast((128, C)))
        bp_sb = pp.tile([128, C // 128], dt.float32, tag="bp")
        nc.gpsimd.dma_start(out=bp_sb[:], in_=bp_ext[:, :])

        dummy_sb = pp.tile([128, QHW], dt.float16, tag="dummy_sb")
        nc.vector.memset(dummy_sb[:].bitcast(dt.float32), 0.0)

        # ---- input loads: ONE queue, priority order. HBM is the wall
        # (~300 GB/s), so what matters is which bytes land first: x^T, then
        # the Q/K columns for head-pair 0 (d3 0:3 / 6:9), then V columns
        # (att0's PV fillers), then the rest, then w_proj.
        xt_all = pp.tile([128, CT, N], dt.float16, tag="xt_all")
        nc.gpsimd.dma_start(
            out=xt_all[:], in_=xt_ext[:, :].rearrange("(i p) n -> p i n", p=128)
        )
        wq_all = pp.tile([128, CT, C3], dt.float16, tag="wq_all")
        for i in range(CT):
            nc.gpsimd.dma_start(
                out=wq_all[:, i, :], in_=wq_ext[128 * i:128 * (i + 1), :]
            )
        wp_all = pp.tile([128, CT, C], dt.float16, tag="wp_all")
        nc.gpsimd.dma_start(
            out=wp_all[:], in_=wp_ext[:, :].rearrange("(i p) n -> p i n", p=128)
        )
        xt_sb = [xt_all[:, i, :] for i in range(CT)]
        wq_sb = [wq_all[:, i, :] for i in range(CT)]
        wp_sb = [wp_all[:, i, :] for i in range(CT)]

        # attn_out^T: 6 persistent tiles of [128, N]
        aT = [pp.tile([128, N], dt.float16, name=f"aT{i}", tag=f"aT{i}") for i in range(CT)]
        # V per token-tile [128, NH, 128] fp16: col 0 = ones (softmax
        # denominator row), cols 1:64 zero, cols 64:128 = V for that head.
        v_sb = [pp.tile([128, NH, 128], dt.float16, name=f"v{i}", tag=f"v{i}") for i in range(TT)]

        # rotating Q^T/K^T tiles, keyed by d3-tile index
        qkT = {}

        def qk_chain_thunks(d3):
            """Build one Q^T/K^T projection chain as a list of thunks (12
            matmuls into two half-bank psum chains, then bias -> qkT[d3]) so
            the matmuls can be sprinkled into the attention PE stream."""
            t = qkrot.tile([128, N], dt.float16, tag="qv" if d3 < 6 else "kv",
                           name=f"qkT{d3}")
            qkT[d3] = t
            pss = [ps_o.tile([128, QHW], dt.float32, tag="pov", name=f"ps_qk{d3}_{qh}")
                   for qh in range(NQH)]
            thunks = []

            def mk_mm(qh, ct):
                def run():
                    nc.tensor.matmul(
                        out=pss[qh][:],
                        lhsT=wq_sb[ct][:, 128 * d3:128 * (d3 + 1)],
                        rhs=xt_sb[ct][:, QHW * qh:QHW * (qh + 1)],
                        start=(ct == 0), stop=(ct == CT - 1),
                    )
                return run

            for qh in range(NQH):
                for ct in range(CT):
                    thunks.append(mk_mm(qh, ct))

            def bias():
                for qh in range(NQH):
                    nc.vector.tensor_scalar(
                        out=t[:, QHW * qh:QHW * (qh + 1)], in0=pss[qh][:],
                        scalar1=bqk_sb[:, d3:d3 + 1], scalar2=None, op0=ALU.add,
                    )
            thunks.append(bias)
            return thunks

        def qk_chain(d3):
            for th in qk_chain_thunks(d3):
                th()

        def att_head(h, fillers=()):
            q_tile = qkT[h // 2]
            k_tile = qkT[6 + h // 2]
            po = 64 * (h % 2)
            fillers = list(fillers)
            # Software-pipelined head: per kt-slot emit fillers, then
            # PV(kt-2), then ST(kt)+exp(kt). ST waits on exp(kt-2)'s PSUM
            # bank; emitting the independent work first keeps the in-order
            # PE queue busy through that wait.
            PIPE = 2
            ess = []
            povs = []
            for qh in range(NQH):
                pov = ps_o.tile([128, QHW], dt.float32, tag="pov", name=f"pov{h}_{qh}")
                povs.append(pov)
            fi = 0
            # 2 kt per slot: the K=64 score matmuls run in the PE's 64x128
            # tiling mode while PV/fillers run 128x128 — batching two kt of
            # STs (and two kt of PVs) per slot halves the mode-switch drains.
            NSLOT = TT // 2 + PIPE

            def pv_pair(kt):
                for qh in range(NQH):
                    nc.tensor.matmul(
                        out=povs[qh][:],
                        lhsT=v_sb[kt][:, h, :],
                        rhs=ess[kt][:, QHW * qh:QHW * (qh + 1)],
                        start=(kt == 0), stop=(kt == TT - 1),
                    )

            for sl in range(NSLOT):
                # fillers first: anything a PV may consume (e.g. att0's
                # V tiles) must be emitted before the PV that reads it
                want = min(len(fillers), -(-((sl + 1) * len(fillers)) // (NSLOT - 1)))
                while fi < want:
                    fillers[fi]()
                    fi += 1
                if sl >= PIPE:
                    pv_pair(2 * (sl - PIPE))
                    pv_pair(2 * (sl - PIPE) + 1)
                if sl < TT // 2:
                    pair = []
                    for kt in (2 * sl, 2 * sl + 1):
                        pss = ps_big.tile([128, N], dt.float32, tag="big", name=f"pss{h}_{kt}")
                        for qh in range(NQH):
                            nc.tensor.matmul(
                                out=pss[:, QHW * qh:QHW * (qh + 1)],
                                lhsT=k_tile[po:po + HD, 128 * kt:128 * (kt + 1)],
                                rhs=q_tile[po:po + HD, QHW * qh:QHW * (qh + 1)],
                                start=True, stop=True,
                            )
                        pair.append(pss)
                    for kt in (2 * sl, 2 * sl + 1):
                        es = att_sb.tile([128, N], dt.float16, tag="es", name=f"es{h}_{kt}")
                        nc.scalar.activation(
                            out=es[:], in_=pair[kt - 2 * sl][:], func=AF.Exp, scale=SCALE
                        )
                        ess.append(es)
            while fi < len(fillers):
                fillers[fi]()
                fi += 1
            # normalize rows 64:128 by reciprocal of denominator row 0
            for qh in range(NQH):
                r_sb = att_small.tile([1, QHW], dt.float32, tag="r")
                nc.vector.reciprocal_approx_fast(out=r_sb[:], in_=povs[qh][0:1, :])
                rb_sb = att_small.tile([HD, QHW], dt.float32, tag="rb")
                nc.gpsimd.partition_broadcast(rb_sb[:], r_sb[:])
                nc.vector.tensor_tensor(
                    out=aT[h // 2][po:po + HD, QHW * qh:QHW * (qh + 1)],
                    in0=povs[qh][64:128, :],
                    in1=rb_sb[:],
                    op=ALU.mult,
                )

        # HAM warm-up: dummy matmuls with no input dependencies
        pwarm = ps_big.tile([128, QHW], dt.float32, tag="big", name="pwarm")
        for _ in range(N_WARMUP_MM):
            nc.tensor.matmul(
                out=pwarm[:], lhsT=dummy_sb[:, 0:128], rhs=dummy_sb[:],
                start=True, stop=True,
            )

        qk_chain(0)   # Q heads 0/1
        qk_chain(6)   # K heads 0/1

        # V part of the qkv projection, as thunk lists
        def v_chain_thunks(tt):
            ps = ps_big.tile([128, N], dt.float32, tag="big", name=f"ps_v{tt}")
            thunks = []

            def mk_mm(c0, c1, ct):
                def run():
                    nc.tensor.matmul(
                        out=ps[:, c0:c1],
                        lhsT=xt_sb[ct][:, 128 * tt:128 * (tt + 1)],
                        rhs=wq_sb[ct][:, 2 * C + c0:2 * C + c1],
                        start=(ct == 0), stop=(ct == CT - 1),
                    )
                return run

            for c0, c1 in ((0, 512), (512, C)):
                for ct in range(CT):
                    thunks.append(mk_mm(c0, c1, ct))

            def finish():
                nc.vector.memset(v_sb[tt][:].bitcast(dt.float32), 0.0)
                nc.vector.tensor_tensor(
                    out=v_sb[tt][:, :, 64:128],
                    in0=ps[:, 0:C].rearrange("p (h d) -> p h d", h=NH),
                    in1=bv_sb[:].rearrange("p (h d) -> p h d", h=NH),
                    op=ALU.add,
                )
                nc.vector.tensor_copy(out=v_sb[tt][:, :, 0:1], in_=ones_f32[:])
            thunks.append(finish)
            return thunks

        # attention heads 0..11 with remaining qkv work sprinkled into
        # each head's PE stream: att0 carries ALL V tiles (the V columns of
        # w_qkv arrive mid-att0); att(h) for h=1..9 carries one Q/K chain
        # (pair j delivered during heads 2j-1 and 2j, consumed from head
        # 2j+2 on)
        filler_plan = {
            0: lambda: [t for tt in range(TT) for t in v_chain_thunks(tt)],
            1: lambda: qk_chain_thunks(1) + qk_chain_thunks(7),
            2: lambda: qk_chain_thunks(2),
            3: lambda: qk_chain_thunks(8),
            4: lambda: qk_chain_thunks(3),
            5: lambda: qk_chain_thunks(9),
            6: lambda: qk_chain_thunks(4),
            7: lambda: qk_chain_thunks(10),
            8: lambda: qk_chain_thunks(5),
            9: lambda: qk_chain_thunks(11),
        }
        for h in range(NH):
            att_head(h, filler_plan[h]() if h in filler_plan else ())

        # ---- output projection (y^T: [c_out, tok]; host untransposes) ----
        for co in range(CT):
            ps = ps_big.tile([128, N], dt.float32, tag="big", name=f"ps_y{co}")
            for qh in range(NQH):
                for ct in range(CT):
                    nc.tensor.matmul(
                        out=ps[:, QHW * qh:QHW * (qh + 1)],
                        lhsT=wp_sb[ct][:, 128 * co:128 * (co + 1)],
                        rhs=aT[ct][:, QHW * qh:QHW * (qh + 1)],
                        start=(ct == 0), stop=(ct == CT - 1),
                    )
            y_sb = y_pool.tile([128, N], dt.float16, tag="y")
            nc.vector.tensor_scalar(
                out=y_sb[:], in0=ps[:],
                scalar1=bp_sb[:, co:co + 1], scalar2=None, op0=ALU.add,
            )
            nc.sync.dma_start(out=y_ext[128 * co:128 * (co + 1), :], in_=y_sb[:])

    nc.compile()
    return nc


_NC_CACHE = {}


def kernel(x, w_qkv, b_qkv, w_proj, b_proj, _trace=False):
    x = np.asarray(x, dtype=np.float32)
    w_qkv = np.asarray(w_qkv, dtype=np.float32)
    b_qkv = np.asarray(b_qkv, dtype=np.float32)
    w_proj = np.asarray(w_proj, dtype=np.float32)
    b_proj = np.asarray(b_proj, dtype=np.float32)

    if "nc" not in _NC_CACHE:
        _NC_CACHE["nc"] = _build_nc()
    nc = _NC_CACHE["nc"]

    # host-side prep (pure layout, no arithmetic)
    # b_qkt: Q/K bias columns laid out per d3-tile: [128, 12]
    b_qkt = np.ascontiguousarray(b_qkv[:2 * C].reshape(2 * C // 128, 128).T)
    w_qkv_h = w_qkv.astype(np.float16)
    w_proj_h = w_proj.astype(np.float16)
    b_v = np.ascontiguousarray(b_qkv[2 * C:].reshape(1, C))
    b_p = np.ascontiguousarray(b_proj.reshape(C // 128, 128).T)

    core_ids = list(range(B))
    in_maps = []
    for b in range(B):
        xt = np.ascontiguousarray(x[b].reshape(N, C).T.astype(np.float16))
        in_maps.append({
            "xt": xt,
            "w_qkv": w_qkv_h,
            "b_qkt": b_qkt,
            "b_v": b_v,
            "w_proj": w_proj_h,
            "b_proj": b_p,
        })

    res = run_bass_kernel_spmd(nc, in_maps, core_ids, trace=_trace)
    if _trace:
        _NC_CACHE["last_result"] = res

    out = np.empty((B, 32, 32, C), dtype=np.float32)
    for b in range(B):
        out[b] = res.results[b]["y"].T.reshape(32, 32, C)
    return out
